# revision 4
# baseline (speedup 1.0000x reference)
"""Trainium2 Bass kernel for a 2-layer IndRNN (adding-problem model).

Model (reference):
    xp = x @ W1.T + b1                      # [T, B, H] input projection
    h1_t = relu(xp_t + u1 * h1_{t-1})       # layer-1 IndRNN (elementwise recurrence)
    h2_t = relu(h1_t @ W2.T + b2 + u2 * h2_{t-1})   # layer-2 IndRNN
    out  = h2_T @ Wf.T + bf                 # [B]

Shapes: B=128, T=4096, I=2, H=256. 8 NeuronCores, data-parallel over batch
(16 batch rows per core), weights replicated, zero inter-core communication.

Algorithm per core
------------------
The relu-scan h_t = max(u*h_{t-1} + a_t, 0) is decomposed exactly into two
`tensor_tensor_scan` instructions plus one subtract (valid for any u, no
rescaling / overflow):

    l'_t = u * l'_{t-1} - a_t          (scan: op0=mult, op1=subtract)
    d_t  = max(u * d_{t-1}, l'_t)      (scan: op0=mult, op1=max)
    h_t  = d_t - l'_t

Proof sketch: with l = -l' the linear scan of a, and d_t := h_t - l_t, the
relu recurrence h_t = max(l_t + u*d_{t-1}, 0) gives d_t = max(u*d_{t-1}, -l_t)
by induction; so h = d - l' exactly.

This turns the sequential time loop into full-tile VectorE scan instructions
(1 elem/cycle/partition) instead of per-timestep instruction issue. Layer-1
states for all T feed one big batched matmul (h1 @ W2.T, run in float32r at
full PE speed), whose output feeds the layer-2 scans the same way. Biases are
folded into the ScalarE PSUM->SBUF copies (activation bias). The final
readout is a tiny PE contraction over partitions.

Layout: scan tiles are [128 partitions = h % 128, C time steps] per
(batch row b, h-half). C=512 (one PSUM bank) chunks, chained through the
scan `initial` operand.
"""

import numpy as np

import concourse.bacc as bacc
import concourse.mybir as mybir
from concourse.tile import TileContext
from concourse.bass_utils import run_bass_kernel_spmd

# Problem constants (hardcoded per harness contract).
B, T, I, H = 128, 4096, 2, 256
NCORES = 8
BL = B // NCORES          # 16 batch rows per core
C = 512                   # timesteps per chunk (= one PSUM bank of fp32)
NCHUNK = T // C
F32 = mybir.dt.float32
F32R = mybir.dt.float32r
AF = mybir.ActivationFunctionType
OP = mybir.AluOpType

_NC_CACHE = {}


def _build_nc():
    """Build the per-core Bass graph (SPMD: same graph on all 8 cores)."""
    nc = bacc.Bacc(None, target_bir_lowering=False)

    x_ext = nc.declare_dram_parameter("x", [BL, T, I], F32R, isOutput=False)
    w1t_ext = nc.declare_dram_parameter("w1t", [I, H], F32R, isOutput=False)
    w2t_ext = nc.declare_dram_parameter("w2t", [H, H], F32R, isOutput=False)
    u1b_ext = nc.declare_dram_parameter("u1b", [2, 128, C], F32, isOutput=False)
    u2b_ext = nc.declare_dram_parameter("u2b", [2, 128, C], F32, isOutput=False)
    b1c_ext = nc.declare_dram_parameter("b1c", [2, 128, 1], F32, isOutput=False)
    b2c_ext = nc.declare_dram_parameter("b2c", [2, 128, 1], F32, isOutput=False)
    wfc_ext = nc.declare_dram_parameter("wfc", [2, 128, 1], F32, isOutput=False)
    bfc_ext = nc.declare_dram_parameter("bfc", [1, 1], F32, isOutput=False)
    out_ext = nc.declare_dram_parameter("out", [1, BL], F32, isOutput=True)

    with TileContext(nc) as tc:
        with (
            tc.tile_pool(name="const", bufs=1) as cpool,
            tc.tile_pool(name="xin", bufs=1) as xpool,
            tc.tile_pool(name="io", bufs=1) as iopool,
            tc.tile_pool(name="scan", bufs=1) as spool,
            tc.tile_pool(name="psum", bufs=1, space="PSUM") as ppool,
        ):
            # ---- constants (loaded once) ----
            w1t = cpool.tile([I, H], F32R)
            nc.sync.dma_start(out=w1t, in_=w1t_ext[:, :])
            w2t = [[cpool.tile([128, 128], F32R, tag=f"w2t{hh}{hl}",
                                name=f"w2t{hh}{hl}")
                    for hl in range(2)] for hh in range(2)]
            for hh in range(2):
                for hl in range(2):
                    nc.sync.dma_start(
                        out=w2t[hh][hl],
                        in_=w2t_ext[hh * 128:(hh + 1) * 128, hl * 128:(hl + 1) * 128])
            u1b, u2b, b1c, b2c, wfc = [], [], [], [], []
            for hh in range(2):
                u1b.append(cpool.tile([128, C], F32, tag=f"u1b{hh}", name=f"u1b{hh}"))
                nc.sync.dma_start(out=u1b[hh], in_=u1b_ext[hh])
                u2b.append(cpool.tile([128, C], F32, tag=f"u2b{hh}", name=f"u2b{hh}"))
                nc.sync.dma_start(out=u2b[hh], in_=u2b_ext[hh])
                b1c.append(cpool.tile([128, 1], F32, tag=f"b1c{hh}", name=f"b1c{hh}"))
                nc.sync.dma_start(out=b1c[hh], in_=b1c_ext[hh])
                b2c.append(cpool.tile([128, 1], F32, tag=f"b2c{hh}", name=f"b2c{hh}"))
                nc.sync.dma_start(out=b2c[hh], in_=b2c_ext[hh])
                wfc.append(cpool.tile([128, 1], F32, tag=f"wfc{hh}", name=f"wfc{hh}"))
                nc.sync.dma_start(out=wfc[hh], in_=wfc_ext[hh])
            bfc = cpool.tile([1, 1], F32)
            nc.sync.dma_start(out=bfc, in_=bfc_ext[:, :])
            # final-state collection tile: columns (hl*BL + b)
            h2f = cpool.tile([128, 2 * BL], F32)

            # ---- main loop: batch-row outer, time-chunk inner ----
            for b in range(BL):
                xT = xpool.tile([I, T], F32R, tag="xT", bufs=2)
                nc.sync.dma_start(out=xT, in_=x_ext[b].transpose([1, 0]))
                lm1p, dl1p, lm2p, dl2p = {}, {}, {}, {}
                for c in range(NCHUNK):
                    tsl = slice(c * C, (c + 1) * C)
                    h1 = {}
                    for hh in range(2):
                        # xp = x @ W1.T  (PE, f32r full speed)
                        pxp = ppool.tile([128, C], F32, tag="xp", bufs=3)
                        nc.tensor.matmul(
                            pxp,
                            lhsT=w1t[:, hh * 128:(hh + 1) * 128],
                            rhs=xT[:, tsl],
                            start=True, stop=True)
                        # +b1, PSUM -> SBUF (ScalarE)
                        xq = iopool.tile([128, C], F32, tag="xq", bufs=6)
                        nc.scalar.activation(xq, pxp, AF.Identity, bias=b1c[hh])
                        # l' scan:  l'_t = u1*l'_{t-1} - xp_t
                        lm1 = spool.tile([128, C], F32, tag="lm1", bufs=4)
                        nc.vector.tensor_tensor_scan(
                            out=lm1, data0=u1b[hh], data1=xq,
                            initial=(0.0 if c == 0 else lm1p[hh][:, C - 1:C]),
                            op0=OP.mult, op1=OP.subtract)
                        # delta scan:  d_t = max(u1*d_{t-1}, l'_t)
                        dl1 = spool.tile([128, C], F32, tag="dl1", bufs=4)
                        nc.vector.tensor_tensor_scan(
                            out=dl1, data0=u1b[hh], data1=lm1,
                            initial=(0.0 if c == 0 else dl1p[hh][:, C - 1:C]),
                            op0=OP.mult, op1=OP.max)
                        # h1 = d - l'
                        h1t = iopool.tile([128, C], F32R, tag="h1", bufs=6)
                        nc.vector.tensor_sub(h1t, dl1, lm1)
                        lm1p[hh], dl1p[hh], h1[hh] = lm1, dl1, h1t
                    for hl in range(2):
                        # zp = h1 @ W2.T (accumulate over both h1 halves)
                        pzp = ppool.tile([128, C], F32, tag="zp", bufs=3)
                        for hh in range(2):
                            nc.tensor.matmul(
                                pzp,
                                lhsT=w2t[hh][hl],
                                rhs=h1[hh],
                                start=(hh == 0), stop=(hh == 1))
                        # +b2, PSUM -> SBUF (ScalarE)
                        az = iopool.tile([128, C], F32, tag="az", bufs=6)
                        nc.scalar.activation(az, pzp, AF.Identity, bias=b2c[hl])
                        lm2 = spool.tile([128, C], F32, tag="lm2", bufs=4)
                        nc.vector.tensor_tensor_scan(
                            out=lm2, data0=u2b[hl], data1=az,
                            initial=(0.0 if c == 0 else lm2p[hl][:, C - 1:C]),
                            op0=OP.mult, op1=OP.subtract)
                        dl2 = spool.tile([128, C], F32, tag="dl2", bufs=4)
                        nc.vector.tensor_tensor_scan(
                            out=dl2, data0=u2b[hl], data1=lm2,
                            initial=(0.0 if c == 0 else dl2p[hl][:, C - 1:C]),
                            op0=OP.mult, op1=OP.max)
                        lm2p[hl], dl2p[hl] = lm2, dl2
                        if c == NCHUNK - 1:
                            # final h2 column for this (b, hl)
                            col = hl * BL + b
                            nc.vector.tensor_sub(
                                h2f[:, col:col + 1],
                                dl2[:, C - 1:C], lm2[:, C - 1:C])

            # ---- readout: out[b] = sum_h2 Wf[h2] * h2f[h2, b] + bf ----
            pro = ppool.tile([1, BL], F32, tag="ro")
            for hl in range(2):
                nc.tensor.matmul(
                    pro, lhsT=wfc[hl], rhs=h2f[:, hl * BL:(hl + 1) * BL],
                    start=(hl == 0), stop=(hl == 1))
            res = iopool.tile([1, BL], F32, tag="res")
            nc.scalar.activation(res, pro, AF.Identity, bias=bfc)
            nc.sync.dma_start(out=out_ext[:, :], in_=res)

    nc.compile()
    return nc


def get_nc():
    if "nc" not in _NC_CACHE:
        _NC_CACHE["nc"] = _build_nc()
    return _NC_CACHE["nc"]


def make_in_maps(x, W1, b1, u1, W2, b2, u2, Wf, bf):
    """Host-side prep: shard x over cores, pre-transpose/tile the weights."""
    x = np.ascontiguousarray(np.asarray(x, dtype=np.float32))
    W1 = np.asarray(W1, np.float32); b1 = np.asarray(b1, np.float32)
    u1 = np.asarray(u1, np.float32); W2 = np.asarray(W2, np.float32)
    b2 = np.asarray(b2, np.float32); u2 = np.asarray(u2, np.float32)
    Wf = np.asarray(Wf, np.float32); bf = np.asarray(bf, np.float32)

    w1t = np.ascontiguousarray(W1.T)                      # [I, H]
    w2t = np.ascontiguousarray(W2.T)                      # [H, H] (w2t[h1, h2])
    u1b = np.ascontiguousarray(
        np.repeat(u1.reshape(2, 128, 1), C, axis=2))      # [2, 128, C]
    u2b = np.ascontiguousarray(np.repeat(u2.reshape(2, 128, 1), C, axis=2))
    b1c = np.ascontiguousarray(b1.reshape(2, 128, 1))
    b2c = np.ascontiguousarray(b2.reshape(2, 128, 1))
    wfc = np.ascontiguousarray(Wf.reshape(2, 128, 1))
    bfc = bf.reshape(1, 1)

    shared = dict(w1t=w1t, w2t=w2t, u1b=u1b, u2b=u2b,
                  b1c=b1c, b2c=b2c, wfc=wfc, bfc=bfc)
    return [dict(shared, x=x[i * BL:(i + 1) * BL]) for i in range(NCORES)]


def kernel(x, W1, b1, u1, W2, b2, u2, Wf, bf):
    nc = get_nc()
    in_maps = make_in_maps(x, W1, b1, u1, W2, b2, u2, Wf, bf)
    res = run_bass_kernel_spmd(nc, in_maps, core_ids=list(range(NCORES)))
    return np.concatenate(
        [res.results[i]["out"].reshape(BL) for i in range(NCORES)])


# revision 5
# speedup vs baseline: 165.7882x; 165.7882x over previous
"""Trainium2 Bass kernel for a 2-layer IndRNN (adding-problem model).

Model (reference):
    xp = x @ W1.T + b1                      # [T, B, H] input projection
    h1_t = relu(xp_t + u1 * h1_{t-1})       # layer-1 IndRNN (elementwise recurrence)
    h2_t = relu(h1_t @ W2.T + b2 + u2 * h2_{t-1})   # layer-2 IndRNN
    out  = h2_T @ Wf.T + bf                 # [B]

Shapes: B=128, T=4096, I=2, H=256. 8 NeuronCores, data-parallel over batch
(16 batch rows per core), weights replicated, zero inter-core communication.

Algorithm per core
------------------
1. The relu-scan h_t = max(u*h_{t-1} + a_t, 0) is decomposed exactly into two
   `tensor_tensor_scan` instructions plus one subtract (valid for any u, no
   rescaling / overflow):

       l'_t = u * l'_{t-1} - a_t          (scan: op0=mult, op1=subtract)
       d_t  = max(u * d_{t-1}, l'_t)      (scan: op0=mult, op1=max)
       h_t  = d_t - l'_t

   (With l = -l' the linear scan of a and d_t := h_t - l_t, the relu
   recurrence h_t = max(l_t + u*d_{t-1}, 0) gives d_t = max(u*d_{t-1}, -l_t)
   by induction.) This turns the sequential time loop into full-tile VectorE
   scan instructions (1 elem/cycle/partition) instead of per-timestep
   instruction issue.

2. Layer-1 states for all T feed one big batched matmul (h1 @ W2.T, float32r
   at full PE speed); its output feeds the layer-2 scans the same way.
   Biases fold into the ScalarE PSUM->SBUF copies (activation bias). The h1
   subtract runs on GpSimd to keep VectorE free for the scans.

3. Only h2 at t=T is needed, and the layer-2 recurrence forgets at rate
   |u2|^k. Host sorts the h2 series by |u2| (permuting W2/b2/u2/Wf rows --
   the final dot product is permutation invariant), so each 128-partition
   half starts its scan at the latest chunk where every series' remaining
   influence is < ~1e-5. The low half typically scans 1 of 8 chunks.

Layout: scan tiles are [128 partitions = series, C=512 time steps] per
(batch row, h-half); chunks chain through the scan `initial` operand.
"""

import math

import numpy as np

import concourse.bacc as bacc
import concourse.mybir as mybir
from concourse.tile import TileContext
from concourse.bass_utils import run_bass_kernel_spmd

# Problem constants (hardcoded per harness contract).
B, T, I, H = 128, 4096, 2, 256
NCORES = 8
BL = B // NCORES          # 16 batch rows per core
C = 512                   # timesteps per chunk (= one PSUM bank of fp32)
NCHUNK = T // C
F32 = mybir.dt.float32
F32R = mybir.dt.float32r
AF = mybir.ActivationFunctionType
OP = mybir.AluOpType
# |u|^K <= 1e-5 relative influence -> safe to zero-init K steps back
LOG_TOL = math.log(1e5)

_NC_CACHE = {}


def _build_nc(c0):
    """Build the per-core Bass graph. c0[hl] = first time-chunk the layer-2
    scan of h2-half hl must process (earlier chunks can't influence h2_T)."""
    nc = bacc.Bacc(None, target_bir_lowering=False)

    x_ext = nc.declare_dram_parameter("x", [BL, T, I], F32R, isOutput=False)
    w1t_ext = nc.declare_dram_parameter("w1t", [I, H], F32R, isOutput=False)
    w2t_ext = nc.declare_dram_parameter("w2t", [H, H], F32R, isOutput=False)
    u1b_ext = nc.declare_dram_parameter("u1b", [2, 128, C], F32, isOutput=False)
    u2b_ext = nc.declare_dram_parameter("u2b", [2, 128, C], F32, isOutput=False)
    b1c_ext = nc.declare_dram_parameter("b1c", [2, 128, 1], F32, isOutput=False)
    b2c_ext = nc.declare_dram_parameter("b2c", [2, 128, 1], F32, isOutput=False)
    wfc_ext = nc.declare_dram_parameter("wfc", [2, 128, 1], F32, isOutput=False)
    bfc_ext = nc.declare_dram_parameter("bfc", [1, 1], F32, isOutput=False)
    out_ext = nc.declare_dram_parameter("out", [1, BL], F32, isOutput=True)

    with TileContext(nc) as tc:
        with (
            tc.tile_pool(name="const", bufs=1) as cpool,
            tc.tile_pool(name="xin", bufs=1) as xpool,
            tc.tile_pool(name="io", bufs=1) as iopool,
            tc.tile_pool(name="scan", bufs=1) as spool,
            tc.tile_pool(name="psum", bufs=1, space="PSUM") as ppool,
        ):
            # ---- constants (loaded once) ----
            w1t = cpool.tile([I, H], F32R)
            nc.sync.dma_start(out=w1t, in_=w1t_ext[:, :])
            w2t = [[cpool.tile([128, 128], F32R, tag=f"w2t{hh}{hl}",
                               name=f"w2t{hh}{hl}")
                    for hl in range(2)] for hh in range(2)]
            for hh in range(2):
                for hl in range(2):
                    nc.sync.dma_start(
                        out=w2t[hh][hl],
                        in_=w2t_ext[hh * 128:(hh + 1) * 128, hl * 128:(hl + 1) * 128])
            u1b, u2b, b1c, b2c, wfc = [], [], [], [], []
            for hh in range(2):
                u1b.append(cpool.tile([128, C], F32, tag=f"u1b{hh}", name=f"u1b{hh}"))
                nc.sync.dma_start(out=u1b[hh], in_=u1b_ext[hh])
                u2b.append(cpool.tile([128, C], F32, tag=f"u2b{hh}", name=f"u2b{hh}"))
                nc.sync.dma_start(out=u2b[hh], in_=u2b_ext[hh])
                b1c.append(cpool.tile([128, 1], F32, tag=f"b1c{hh}", name=f"b1c{hh}"))
                nc.sync.dma_start(out=b1c[hh], in_=b1c_ext[hh])
                b2c.append(cpool.tile([128, 1], F32, tag=f"b2c{hh}", name=f"b2c{hh}"))
                nc.sync.dma_start(out=b2c[hh], in_=b2c_ext[hh])
                wfc.append(cpool.tile([128, 1], F32, tag=f"wfc{hh}", name=f"wfc{hh}"))
                nc.sync.dma_start(out=wfc[hh], in_=wfc_ext[hh])
            bfc = cpool.tile([1, 1], F32)
            nc.sync.dma_start(out=bfc, in_=bfc_ext[:, :])
            # final-state collection tile: columns (hl*BL + b)
            h2f = cpool.tile([128, 2 * BL], F32)

            # ---- main loop: batch-row outer, time-chunk inner ----
            for b in range(BL):
                xT = xpool.tile([I, T], F32R, tag="xT", bufs=2)
                nc.sync.dma_start(out=xT, in_=x_ext[b].transpose([1, 0]))
                lm1p, dl1p, lm2p, dl2p = {}, {}, {}, {}
                for c in range(NCHUNK):
                    tsl = slice(c * C, (c + 1) * C)
                    h1 = {}
                    for hh in range(2):
                        # xp = x @ W1.T  (PE, f32r full speed)
                        pxp = ppool.tile([128, C], F32, tag="xp", bufs=3)
                        nc.tensor.matmul(
                            pxp,
                            lhsT=w1t[:, hh * 128:(hh + 1) * 128],
                            rhs=xT[:, tsl],
                            start=True, stop=True)
                        # +b1, PSUM -> SBUF (ScalarE)
                        xq = iopool.tile([128, C], F32, tag="xq", bufs=6)
                        nc.scalar.activation(xq, pxp, AF.Identity, bias=b1c[hh])
                        # l' scan:  l'_t = u1*l'_{t-1} - xp_t
                        lm1 = spool.tile([128, C], F32, tag="lm1", bufs=4)
                        nc.vector.tensor_tensor_scan(
                            out=lm1, data0=u1b[hh], data1=xq,
                            initial=(0.0 if c == 0 else lm1p[hh][:, C - 1:C]),
                            op0=OP.mult, op1=OP.subtract)
                        # delta scan:  d_t = max(u1*d_{t-1}, l'_t)
                        dl1 = spool.tile([128, C], F32, tag="dl1", bufs=4)
                        nc.vector.tensor_tensor_scan(
                            out=dl1, data0=u1b[hh], data1=lm1,
                            initial=(0.0 if c == 0 else dl1p[hh][:, C - 1:C]),
                            op0=OP.mult, op1=OP.max)
                        # h1 = d - l'  (GpSimd -- keeps VectorE free for scans)
                        h1t = iopool.tile([128, C], F32R, tag="h1", bufs=6)
                        nc.gpsimd.tensor_sub(h1t, dl1, lm1)
                        lm1p[hh], dl1p[hh], h1[hh] = lm1, dl1, h1t
                    for hl in range(2):
                        if c < c0[hl]:
                            continue
                        # zp = h1 @ W2.T (accumulate over both h1 halves)
                        pzp = ppool.tile([128, C], F32, tag="zp", bufs=3)
                        for hh in range(2):
                            nc.tensor.matmul(
                                pzp,
                                lhsT=w2t[hh][hl],
                                rhs=h1[hh],
                                start=(hh == 0), stop=(hh == 1))
                        # +b2, PSUM -> SBUF (ScalarE)
                        az = iopool.tile([128, C], F32, tag="az", bufs=6)
                        nc.scalar.activation(az, pzp, AF.Identity, bias=b2c[hl])
                        lm2 = spool.tile([128, C], F32, tag="lm2", bufs=4)
                        nc.vector.tensor_tensor_scan(
                            out=lm2, data0=u2b[hl], data1=az,
                            initial=(0.0 if c == c0[hl] else lm2p[hl][:, C - 1:C]),
                            op0=OP.mult, op1=OP.subtract)
                        dl2 = spool.tile([128, C], F32, tag="dl2", bufs=4)
                        nc.vector.tensor_tensor_scan(
                            out=dl2, data0=u2b[hl], data1=lm2,
                            initial=(0.0 if c == c0[hl] else dl2p[hl][:, C - 1:C]),
                            op0=OP.mult, op1=OP.max)
                        lm2p[hl], dl2p[hl] = lm2, dl2
                        if c == NCHUNK - 1:
                            # final h2 column for this (b, hl)
                            col = hl * BL + b
                            nc.gpsimd.tensor_sub(
                                h2f[:, col:col + 1],
                                dl2[:, C - 1:C], lm2[:, C - 1:C])

            # ---- readout: out[b] = sum_h2 Wf[h2] * h2f[h2, b] + bf ----
            pro = ppool.tile([1, BL], F32, tag="ro")
            for hl in range(2):
                nc.tensor.matmul(
                    pro, lhsT=wfc[hl], rhs=h2f[:, hl * BL:(hl + 1) * BL],
                    start=(hl == 0), stop=(hl == 1))
            res = iopool.tile([1, BL], F32, tag="res")
            nc.scalar.activation(res, pro, AF.Identity, bias=bfc)
            nc.sync.dma_start(out=out_ext[:, :], in_=res)

    nc.compile()
    return nc


def _chunk_starts(u2s):
    """First chunk each sorted h2-half must scan: |u|^K < 1e-5 horizon."""
    c0 = []
    for hl in range(2):
        grp = np.abs(u2s[hl * 128:(hl + 1) * 128])
        umax = float(grp.max())
        if umax >= math.exp(-LOG_TOL / T):      # needs (almost) full history
            k = T
        else:
            k = min(T, int(math.ceil(LOG_TOL / -math.log(max(umax, 1e-30)))))
        c0.append(NCHUNK - (k + C - 1) // C)
    return tuple(c0)


def prepare(x, W1, b1, u1, W2, b2, u2, Wf, bf):
    """Host-side prep: shard x, permute h2 series by |u2|, tile weights.
    Returns (nc, in_maps)."""
    x = np.ascontiguousarray(np.asarray(x, dtype=np.float32))
    W1 = np.asarray(W1, np.float32); b1 = np.asarray(b1, np.float32)
    u1 = np.asarray(u1, np.float32); W2 = np.asarray(W2, np.float32)
    b2 = np.asarray(b2, np.float32); u2 = np.asarray(u2, np.float32)
    Wf = np.asarray(Wf, np.float32); bf = np.asarray(bf, np.float32)

    # sort h2 series by |u2| so truncation is per-128-half (output is a
    # permutation-invariant sum over h2)
    pi2 = np.argsort(np.abs(u2), kind="stable")
    u2s = u2[pi2]
    c0 = _chunk_starts(u2s)

    w1t = np.ascontiguousarray(W1.T)                      # [I, H]
    w2t = np.ascontiguousarray(W2.T[:, pi2])              # [h1, h2-sorted]
    u1b = np.ascontiguousarray(np.repeat(u1.reshape(2, 128, 1), C, axis=2))
    u2b = np.ascontiguousarray(np.repeat(u2s.reshape(2, 128, 1), C, axis=2))
    b1c = np.ascontiguousarray(b1.reshape(2, 128, 1))
    b2c = np.ascontiguousarray(b2[pi2].reshape(2, 128, 1))
    wfc = np.ascontiguousarray(Wf.reshape(-1)[pi2].reshape(2, 128, 1))
    bfc = bf.reshape(1, 1)

    if c0 not in _NC_CACHE:
        _NC_CACHE[c0] = _build_nc(c0)
    nc = _NC_CACHE[c0]

    shared = dict(w1t=w1t, w2t=w2t, u1b=u1b, u2b=u2b,
                  b1c=b1c, b2c=b2c, wfc=wfc, bfc=bfc)
    in_maps = [dict(shared, x=x[i * BL:(i + 1) * BL]) for i in range(NCORES)]
    return nc, in_maps


def kernel(x, W1, b1, u1, W2, b2, u2, Wf, bf):
    nc, in_maps = prepare(x, W1, b1, u1, W2, b2, u2, Wf, bf)
    res = run_bass_kernel_spmd(nc, in_maps, core_ids=list(range(NCORES)))
    return np.concatenate(
        [res.results[i]["out"].reshape(BL) for i in range(NCORES)])
